# revision 12
# baseline (speedup 1.0000x reference)
"""GRU (Flax GRUCell scanned over time) on 8 Trainium2 NeuronCores.

Problem: x:[T,B,D]=[512,64,512], h0:[B,H], Wi:[D,3H], Wh:[H,3H], bi:[3H], bhn:[H]
  gi = x_t @ Wi + bi ; gh = h @ Wh ; gates (r,z,n); h' = (1-z)*n + z*h
  returns ys:[T,B,H] (the h trajectory).

Strategy (per core, data-parallel over batch, B_local=8):
  T-layout on chip: hidden dim on SBUF partitions, batch on the free dim.

  v2: the input transform giT = Wi.T @ xT is batched over RT2=16 timesteps
  into a PSUM-resident chunk [128, M3, RT2*BL] (3 banks, double-buffered):
  one N=128 matmul per (m,k) weight tile instead of 16 N=8 matmuls. This
  halves the PE instruction stream (each matmul needs its own LDWEIGHTS,
  and LDWEIGHTS of the 128x128 tile dominates at N=8).

  Per step, the recurrent ghT = Wh.T @ hT matmuls accumulate r/z rows
  directly ON TOP of the chunk's r/z columns for that step (so sigmoid
  reads gi+gh straight from PSUM); n rows go to their own PSUM tile so
  r can gate gh_n alone. The next chunk's gi matmuls are emitted 4-per-
  step, filling the PE idle window while the gate elementwise chain runs.

  Gate math per step (critical chain):
    sigmoid(psum rz) -> rpn -> pre_n -> tanh -> w -> hb(bf16)
  with omz = 1-z, v = z*h and the fp32 h' off-chain on GpSimd.
  h state stays fp32; output is written transposed and reassembled on host.
"""

import warnings

warnings.filterwarnings("ignore")

import numpy as np
import ml_dtypes

import concourse.bacc as bacc
import concourse.tile as tile
from concourse import mybir, bass_utils

B, D, H = 64, 512, 512
NCORES = 8
BL = B // NCORES  # batch per core
KD = D // 128  # input-dim k-chunks
KH = H // 128  # hidden-dim k-chunks
M3 = 3 * H // 128  # 3H m-tiles
RT = 8  # output-ring steps per DMA
RT2 = 16  # gi chunk timesteps (chunk = 3 PSUM banks)
BF16 = mybir.dt.bfloat16
F32 = mybir.dt.float32
NPBF16 = ml_dtypes.bfloat16

_cache: dict = {}


def _build(T: int, use_bi: bool, use_bhn: bool):
    TB = T * BL
    assert T % RT == 0 and T % RT2 == 0
    NCH = T // RT2
    CHW = RT2 * BL  # chunk width in fp32 columns (=128)
    nc = bacc.Bacc("TRN2", target_bir_lowering=False, debug=False, num_devices=NCORES)

    xt_d = nc.dram_tensor("xt", [128, KD * TB], BF16, kind="ExternalInput").ap()
    wi_d = nc.dram_tensor("wi", [128, M3 * KD * 128], BF16, kind="ExternalInput").ap()
    wh_d = nc.dram_tensor("wh", [128, M3 * KH * 128], BF16, kind="ExternalInput").ap()
    h0_d = nc.dram_tensor("h0t", [128, KH * BL], F32, kind="ExternalInput").ap()
    bi_d = (
        nc.dram_tensor("bi_r", [1, M3 * 128], BF16, kind="ExternalInput").ap()
        if use_bi
        else None
    )
    bhn_d = (
        nc.dram_tensor("bhn_t", [128, KH], F32, kind="ExternalInput").ap()
        if use_bhn
        else None
    )
    ys_d = nc.dram_tensor("yst", [128, KH * TB], F32, kind="ExternalOutput").ap()
    ys_v = ys_d.rearrange("p (k t j) -> p k t j", k=KH, j=BL)

    with tile.TileContext(nc) as tc:
        with (
            tc.tile_pool(name="const", bufs=1) as const,
            tc.tile_pool(name="xin", bufs=1) as xin,
            tc.tile_pool(name="chk", bufs=2, space="PSUM") as chk,
            tc.tile_pool(name="whps", bufs=2, space="PSUM") as whps,
            tc.tile_pool(name="orp", bufs=3) as orp,
            tc.tile_pool(name="hbp", bufs=2) as hbp,
            tc.tile_pool(name="ew", bufs=2) as ew,
            tc.tile_pool(name="ginp", bufs=2) as ginp,
        ):
            # ---- load constants ----
            wi_sb = const.tile([128, M3 * KD * 128], BF16)
            nc.sync.dma_start(wi_sb[:], wi_d[:])
            wh_sb = const.tile([128, M3 * KH * 128], BF16)
            nc.sync.dma_start(wh_sb[:], wh_d[:])
            h0_sb = const.tile([128, KH, BL], F32)
            nc.sync.dma_start(h0_sb[:], h0_d.rearrange("p (k j) -> p k j", j=BL))
            if use_bi:
                bi_sb = const.tile([1, M3 * 128], BF16)
                nc.sync.dma_start(bi_sb[:], bi_d[:])
                ones_sb = const.tile([1, CHW], BF16)
                nc.vector.memset(ones_sb[:], 1.0)
            if use_bhn:
                bhn_sb = const.tile([128, KH], F32)
                nc.sync.dma_start(bhn_sb[:], bhn_d[:])
            xt_sb = xin.tile([128, KD * TB], BF16)
            nc.sync.dma_start(xt_sb[:], xt_d[:])

            def gi_chunk_mm(gct, c, j):
                """j-th (0..47) gi matmul of chunk c: one (m,k) weight tile
                with all RT2 timesteps as the moving operand. r/z banks are
                left open (no stop): per-step Wh matmuls accumulate on top.
                start=True only on the first matmul touching each PSUM bank
                (the lazy zero covers the whole bank)."""
                m, k = j // KD, j % KD
                t0 = c * RT2
                nc.tensor.matmul(
                    gct[:, m, :],
                    wi_sb[:, (m * KD + k) * 128 : (m * KD + k + 1) * 128],
                    xt_sb[:, k * TB + t0 * BL : k * TB + t0 * BL + CHW],
                    start=(k == 0 and m % 4 == 0),
                    stop=(k == KD - 1) and (m >= 8) and not use_bi,
                    skip_group_check=True,
                )
                if use_bi and k == KD - 1:
                    nc.tensor.matmul(
                        gct[:, m, :],
                        bi_sb[:, m * 128 : (m + 1) * 128],
                        ones_sb[:],
                        start=False,
                        stop=(m >= 8),
                        skip_group_check=True,
                    )

            # ---- prologue: chunk 0 in full ----
            cur = chk.tile([128, M3, CHW], F32, tag="gchunk")
            for j in range(KD * M3):
                gi_chunk_mm(cur, 0, j)
            # gi_n rows copied PSUM->SBUF once per chunk (off the critical
            # chain) so pre_n reads SBUF instead of PSUM
            cur_n = ginp.tile([128, KH, CHW], F32, tag="ginsb")
            nc.scalar.copy(cur_n[:], cur[:, 8:12, :])

            hb = hbp.tile([128, KH, BL], BF16, tag="hb")
            nc.vector.tensor_copy(hb[:], h0_sb[:])
            h_prev = h0_sb[:, :, :]

            o_cur = None
            nxt = None
            for t in range(T):
                u = t % RT2
                c = t // RT2
                ur = t % RT
                if ur == 0:
                    o_cur = orp.tile([128, KH, RT, BL], F32, tag="oring")
                if u == 0:
                    nxt = (
                        chk.tile([128, M3, CHW], F32, tag="gchunk", name="gct")
                        if c + 1 < NCH
                        else None
                    )
                cl = u * BL  # this step's column offset in the chunk

                # on-chain: ghT matmuls (need h from last step).
                # r/z rows accumulate into the chunk psum (on top of gi);
                # n rows into their own psum so r can gate gh_n alone.
                ps = whps.tile([128, KH, BL], F32, tag="whp")
                for k in range(KH):  # k-outer: each pass needs only hb[:,k,:]
                    for m in range(M3):
                        out_ap = (
                            cur[:, m, cl : cl + BL]
                            if m < 8
                            else ps[:, m - 8, :]
                        )
                        nc.tensor.matmul(
                            out_ap,
                            wh_sb[:, (m * KH + k) * 128 : (m * KH + k + 1) * 128],
                            hb[:, k, :],
                            start=(m == 8 and k == 0),
                            stop=(k == KH - 1),
                            skip_group_check=True,
                        )
                # off-chain: next chunk's giT matmuls fill the PE idle window
                if nxt is not None and 1 <= u <= 12:
                    for j in range((u - 1) * KD, u * KD):
                        gi_chunk_mm(nxt, c + 1, j)
                if nxt is not None and u == 13:
                    nxt_n = ginp.tile([128, KH, CHW], F32, tag="ginsb")
                    nc.scalar.copy(nxt_n[:], nxt[:, 8:12, :])

                # Gate math, critical chain split by k-chunk (k0 lane first
                # so hb[:,0,:] unblocks the next step's first matmul pass):
                #   sigmoid(psum rz) -> rpn0 -> pren0 -> tanh0 -> w0 -> hb0
                # Off-chain on GpSimd: omz = 1-z, v = z*h_prev, fp32 h'.
                # h' = (1-z)*n + z*h = omz*n + v
                rzt = ew.tile([128, 8, BL], F32, tag="rzt")
                nc.scalar.activation(
                    rzt[:],
                    cur[:, 0:8, cl : cl + BL],
                    mybir.ActivationFunctionType.Sigmoid,
                )
                omz = ew.tile([128, KH, BL], F32, tag="omz")
                nc.gpsimd.tensor_scalar(
                    omz[:],
                    rzt[:, KH : 2 * KH, :],
                    -1.0,
                    1.0,
                    mybir.AluOpType.mult,
                    mybir.AluOpType.add,
                )
                v = ew.tile([128, KH, BL], F32, tag="v")
                nc.gpsimd.tensor_mul(v[:], rzt[:, KH : 2 * KH, :], h_prev)
                rpn = ew.tile([128, KH, BL], F32, tag="rpn")
                pre_n = ew.tile([128, KH, BL], F32, tag="pren")
                nt = ew.tile([128, KH, BL], F32, tag="nt")
                w = ew.tile([128, KH, BL], F32, tag="w")
                hb = hbp.tile([128, KH, BL], BF16, tag="hb")
                if use_bhn:
                    for k in range(KH):
                        nc.vector.scalar_tensor_tensor(
                            rpn[:, k, :],
                            ps[:, k, :],
                            bhn_sb[:, k : k + 1],
                            rzt[:, k, :],
                            mybir.AluOpType.add,
                            mybir.AluOpType.mult,
                        )
                else:
                    nc.vector.tensor_mul(rpn[:, 0, :], ps[:, 0, :], rzt[:, 0, :])
                    nc.vector.tensor_mul(
                        rpn[:, 1:4, :], ps[:, 1:4, :], rzt[:, 1:4, :]
                    )
                nc.vector.tensor_add(
                    pre_n[:, 0, :], rpn[:, 0, :], cur_n[:, 0, cl : cl + BL]
                )
                nc.scalar.activation(
                    nt[:, 0, :], pre_n[:, 0, :], mybir.ActivationFunctionType.Tanh
                )
                nc.vector.tensor_add(
                    pre_n[:, 1:4, :], rpn[:, 1:4, :], cur_n[:, 1:4, cl : cl + BL]
                )
                nc.vector.tensor_mul(w[:, 0, :], nt[:, 0, :], omz[:, 0, :])
                if t + 1 < T:
                    nc.vector.tensor_add(hb[:, 0, :], w[:, 0, :], v[:, 0, :])
                nc.scalar.activation(
                    nt[:, 1:4, :],
                    pre_n[:, 1:4, :],
                    mybir.ActivationFunctionType.Tanh,
                )
                nc.vector.tensor_mul(w[:, 1:4, :], nt[:, 1:4, :], omz[:, 1:4, :])
                if t + 1 < T:
                    nc.vector.tensor_add(hb[:, 1:4, :], w[:, 1:4, :], v[:, 1:4, :])
                h_new = o_cur[:, :, ur, :]
                # fp32 h for output/next-step v, off the critical chain
                nc.gpsimd.tensor_add(h_new, w[:], v[:])
                h_prev = h_new

                if u == RT2 - 1:
                    cur = nxt
                    cur_n = nxt_n

                if ur == RT - 1:
                    nc.sync.dma_start(
                        ys_v[:, :, t - RT + 1 : t + 1, :], o_cur[:]
                    )

    nc.compile()
    return nc


def _get(T, use_bi, use_bhn):
    key = (T, use_bi, use_bhn)
    if key not in _cache:
        _cache[key] = _build(T, use_bi, use_bhn)
    return _cache[key]


def _pack_w(W, kc):
    # W [kc*128, M3*128] -> [128, M3*kc*128], col ((m*kc)+k)*128+c = W[k*128+p, m*128+c]
    return np.ascontiguousarray(
        W.astype(NPBF16).reshape(kc, 128, M3, 128).transpose(1, 2, 0, 3).reshape(128, -1)
    )


def kernel(x, h0, Wi, Wh, bi, bhn, _trace=False, _trace_kwargs=None):
    T = x.shape[0]
    use_bi = bool(np.any(bi))
    use_bhn = bool(np.any(bhn))
    nc = _get(T, use_bi, use_bhn)
    TB = T * BL

    wi_p = _pack_w(np.asarray(Wi), KD)
    wh_p = _pack_w(np.asarray(Wh), KH)
    x = np.asarray(x)
    h0 = np.asarray(h0)

    in_maps = []
    for c in range(NCORES):
        xc = x[:, c * BL : (c + 1) * BL, :]  # [T, BL, D]
        xt = np.ascontiguousarray(
            xc.astype(NPBF16).reshape(T, BL, KD, 128).transpose(3, 2, 0, 1).reshape(128, KD * TB)
        )
        h0c = np.ascontiguousarray(
            h0[c * BL : (c + 1) * BL, :].astype(np.float32).reshape(BL, KH, 128).transpose(2, 1, 0).reshape(128, KH * BL)
        )
        im = {"xt": xt, "wi": wi_p, "wh": wh_p, "h0t": h0c}
        if use_bi:
            im["bi_r"] = np.ascontiguousarray(bi.astype(NPBF16).reshape(1, M3 * 128))
        if use_bhn:
            im["bhn_t"] = np.ascontiguousarray(bhn.astype(np.float32).reshape(KH, 128).T)
        in_maps.append(im)

    kw = {}
    if _trace:
        kw = dict(trace=True, **(_trace_kwargs or {}))
    kernel._last_in_maps = in_maps
    res = bass_utils.run_bass_kernel_spmd(nc, in_maps, core_ids=list(range(NCORES)), **kw)

    ys = np.empty((T, B, H), dtype=np.float32)
    for c in range(NCORES):
        out = res.results[c]["yst"]  # [128, KH*TB]
        ys[:, c * BL : (c + 1) * BL, :] = (
            out.reshape(128, KH, T, BL).transpose(2, 3, 1, 0).reshape(T, BL, H)
        )
    kernel._last_result = res
    return ys


# revision 13
# speedup vs baseline: 1.0356x; 1.0356x over previous
"""GRU (Flax GRUCell scanned over time) on 8 Trainium2 NeuronCores.

Problem: x:[T,B,D]=[512,64,512], h0:[B,H], Wi:[D,3H], Wh:[H,3H], bi:[3H], bhn:[H]
  gi = x_t @ Wi + bi ; gh = h @ Wh ; gates (r,z,n); h' = (1-z)*n + z*h
  returns ys:[T,B,H] (the h trajectory).

Strategy (per core, data-parallel over batch, B_local=8):
  T-layout on chip: hidden dim on SBUF partitions, batch on the free dim.

  v2: the input transform giT = Wi.T @ xT is batched over RT2=16 timesteps
  into a PSUM-resident chunk [128, M3, RT2*BL] (3 banks, double-buffered):
  one N=128 matmul per (m,k) weight tile instead of 16 N=8 matmuls. This
  halves the PE instruction stream (each matmul needs its own LDWEIGHTS,
  and LDWEIGHTS of the 128x128 tile dominates at N=8).

  Per step, the recurrent ghT = Wh.T @ hT matmuls accumulate r/z rows
  directly ON TOP of the chunk's r/z columns for that step (so sigmoid
  reads gi+gh straight from PSUM); n rows go to their own PSUM tile so
  r can gate gh_n alone. The next chunk's gi matmuls are emitted 4-per-
  step, filling the PE idle window while the gate elementwise chain runs.

  Gate math per step (critical chain):
    sigmoid(psum rz) -> rpn -> pre_n -> tanh -> w -> hb(bf16)
  with omz = 1-z, v = z*h and the fp32 h' off-chain on GpSimd.
  h state stays fp32; output is written transposed and reassembled on host.
"""

import warnings

warnings.filterwarnings("ignore")

import numpy as np
import ml_dtypes

import concourse.bacc as bacc
import concourse.tile as tile
from concourse import mybir, bass_utils

B, D, H = 64, 512, 512
NCORES = 8
BL = B // NCORES  # batch per core
KD = D // 128  # input-dim k-chunks
KH = H // 128  # hidden-dim k-chunks
M3 = 3 * H // 128  # 3H m-tiles
RT = 8  # output-ring steps per DMA
RT2 = 16  # gi chunk timesteps (chunk = 3 PSUM banks)
BF16 = mybir.dt.bfloat16
F32 = mybir.dt.float32
NPBF16 = ml_dtypes.bfloat16

_cache: dict = {}


def _build(T: int, use_bi: bool, use_bhn: bool):
    TB = T * BL
    assert T % RT == 0 and T % RT2 == 0
    NCH = T // RT2
    CHW = RT2 * BL  # chunk width in fp32 columns (=128)
    nc = bacc.Bacc("TRN2", target_bir_lowering=False, debug=False, num_devices=NCORES)

    xt_d = nc.dram_tensor("xt", [128, KD * TB], BF16, kind="ExternalInput").ap()
    wi_d = nc.dram_tensor("wi", [128, M3 * KD * 128], BF16, kind="ExternalInput").ap()
    wh_d = nc.dram_tensor("wh", [128, M3 * KH * 128], BF16, kind="ExternalInput").ap()
    h0_d = nc.dram_tensor("h0t", [128, KH * BL], F32, kind="ExternalInput").ap()
    bi_d = (
        nc.dram_tensor("bi_r", [1, M3 * 128], BF16, kind="ExternalInput").ap()
        if use_bi
        else None
    )
    bhn_d = (
        nc.dram_tensor("bhn_t", [128, KH], F32, kind="ExternalInput").ap()
        if use_bhn
        else None
    )
    ys_d = nc.dram_tensor("yst", [128, KH * TB], F32, kind="ExternalOutput").ap()
    ys_v = ys_d.rearrange("p (k t j) -> p k t j", k=KH, j=BL)

    with tile.TileContext(nc) as tc:
        with (
            tc.tile_pool(name="const", bufs=1) as const,
            tc.tile_pool(name="xin", bufs=1) as xin,
            tc.tile_pool(name="chk", bufs=2, space="PSUM") as chk,
            tc.tile_pool(name="whps", bufs=2, space="PSUM") as whps,
            tc.tile_pool(name="orp", bufs=3) as orp,
            tc.tile_pool(name="hbp", bufs=2) as hbp,
            tc.tile_pool(name="ew", bufs=2) as ew,
            tc.tile_pool(name="ginp", bufs=2) as ginp,
        ):
            # ---- load constants ----
            wi_sb = const.tile([128, M3 * KD * 128], BF16)
            nc.sync.dma_start(wi_sb[:], wi_d[:])
            wh_sb = const.tile([128, M3 * KH * 128], BF16)
            nc.sync.dma_start(wh_sb[:], wh_d[:])
            h0_sb = const.tile([128, KH, BL], F32)
            nc.sync.dma_start(h0_sb[:], h0_d.rearrange("p (k j) -> p k j", j=BL))
            if use_bi:
                bi_sb = const.tile([1, M3 * 128], BF16)
                nc.sync.dma_start(bi_sb[:], bi_d[:])
                ones_sb = const.tile([1, CHW], BF16)
                nc.vector.memset(ones_sb[:], 1.0)
            if use_bhn:
                bhn_sb = const.tile([128, KH], F32)
                nc.sync.dma_start(bhn_sb[:], bhn_d[:])
            xt_sb = xin.tile([128, KD * TB], BF16)
            nc.sync.dma_start(xt_sb[:], xt_d[:])

            def gi_chunk_mm(gct, c, j):
                """j-th (0..47) gi matmul of chunk c: one (m,k) weight tile
                with all RT2 timesteps as the moving operand. r/z banks are
                left open (no stop): per-step Wh matmuls accumulate on top.
                start=True only on the first matmul touching each PSUM bank
                (the lazy zero covers the whole bank)."""
                m, k = j // KD, j % KD
                t0 = c * RT2
                nc.tensor.matmul(
                    gct[:, m, :],
                    wi_sb[:, (m * KD + k) * 128 : (m * KD + k + 1) * 128],
                    xt_sb[:, k * TB + t0 * BL : k * TB + t0 * BL + CHW],
                    start=(k == 0 and m % 4 == 0),
                    stop=(k == KD - 1) and (m >= 8) and not use_bi,
                    skip_group_check=True,
                )
                if use_bi and k == KD - 1:
                    nc.tensor.matmul(
                        gct[:, m, :],
                        bi_sb[:, m * 128 : (m + 1) * 128],
                        ones_sb[:],
                        start=False,
                        stop=(m >= 8),
                        skip_group_check=True,
                    )

            # ---- prologue: chunk 0 in full ----
            cur = chk.tile([128, M3, CHW], F32, tag="gchunk")
            for j in range(KD * M3):
                gi_chunk_mm(cur, 0, j)
            # gi_n rows copied PSUM->SBUF once per chunk (off the critical
            # chain) so pre_n reads SBUF instead of PSUM
            cur_n = ginp.tile([128, KH, CHW], F32, tag="ginsb")
            nc.scalar.copy(cur_n[:], cur[:, 8:12, :])

            hb = hbp.tile([128, KH, BL], BF16, tag="hb")
            nc.vector.tensor_copy(hb[:], h0_sb[:])
            h_prev = h0_sb[:, :, :]

            o_cur = None
            nxt = None
            for t in range(T):
                u = t % RT2
                c = t // RT2
                ur = t % RT
                if ur == 0:
                    o_cur = orp.tile([128, KH, RT, BL], F32, tag="oring")
                if u == 0:
                    nxt = (
                        chk.tile([128, M3, CHW], F32, tag="gchunk", name="gct")
                        if c + 1 < NCH
                        else None
                    )
                cl = u * BL  # this step's column offset in the chunk

                # on-chain: ghT matmuls (need h from last step).
                # r/z rows accumulate into the chunk psum (on top of gi);
                # n rows into their own psum so r can gate gh_n alone.
                ps = whps.tile([128, KH, BL], F32, tag="whp")
                for k in range(KH):  # k-outer: each pass needs only hb[:,k,:]
                    for m in range(M3):
                        out_ap = (
                            cur[:, m, cl : cl + BL]
                            if m < 8
                            else ps[:, m - 8, :]
                        )
                        nc.tensor.matmul(
                            out_ap,
                            wh_sb[:, (m * KH + k) * 128 : (m * KH + k + 1) * 128],
                            hb[:, k, :],
                            start=(m == 8 and k == 0),
                            stop=(k == KH - 1),
                            skip_group_check=True,
                        )
                # off-chain: next chunk's giT matmuls fill the PE idle window
                if nxt is not None and 1 <= u <= 12:
                    for j in range((u - 1) * KD, u * KD):
                        gi_chunk_mm(nxt, c + 1, j)
                if nxt is not None and u == 13:
                    nxt_n = ginp.tile([128, KH, CHW], F32, tag="ginsb")
                    nc.scalar.copy(nxt_n[:], nxt[:, 8:12, :])

                # Gate math, critical chain split by k-chunk (k0 lane first
                # so hb[:,0,:] unblocks the next step's first matmul pass):
                #   sigmoid(psum rz) -> rpn0 -> pren0 -> tanh0 -> w0 -> hb0
                # Off-chain on GpSimd: omz = 1-z, v = z*h_prev, fp32 h'.
                # h' = (1-z)*n + z*h = omz*n + v
                rzt = ew.tile([128, 8, BL], F32, tag="rzt")
                nc.scalar.activation(
                    rzt[:],
                    cur[:, 0:8, cl : cl + BL],
                    mybir.ActivationFunctionType.Sigmoid,
                )
                omz = ew.tile([128, KH, BL], F32, tag="omz")
                nc.gpsimd.tensor_scalar(
                    omz[:],
                    rzt[:, KH : 2 * KH, :],
                    -1.0,
                    1.0,
                    mybir.AluOpType.mult,
                    mybir.AluOpType.add,
                )
                v = ew.tile([128, KH, BL], F32, tag="v")
                nc.gpsimd.tensor_mul(v[:], rzt[:, KH : 2 * KH, :], h_prev)
                rpn = ew.tile([128, KH, BL], F32, tag="rpn")
                pre_n = ew.tile([128, KH, BL], F32, tag="pren")
                nt = ew.tile([128, KH, BL], F32, tag="nt")
                w = ew.tile([128, KH, BL], F32, tag="w")
                hb = hbp.tile([128, KH, BL], BF16, tag="hb")
                if use_bhn:
                    for k in range(KH):
                        nc.vector.scalar_tensor_tensor(
                            rpn[:, k, :],
                            ps[:, k, :],
                            bhn_sb[:, k : k + 1],
                            rzt[:, k, :],
                            mybir.AluOpType.add,
                            mybir.AluOpType.mult,
                        )
                else:
                    nc.vector.tensor_mul(rpn[:, 0, :], ps[:, 0, :], rzt[:, 0, :])
                nc.vector.tensor_add(
                    pre_n[:, 0, :], rpn[:, 0, :], cur_n[:, 0, cl : cl + BL]
                )
                nc.scalar.activation(
                    nt[:, 0, :], pre_n[:, 0, :], mybir.ActivationFunctionType.Tanh
                )
                if not use_bhn:
                    nc.vector.tensor_mul(
                        rpn[:, 1:4, :], ps[:, 1:4, :], rzt[:, 1:4, :]
                    )
                nc.vector.tensor_add(
                    pre_n[:, 1:4, :], rpn[:, 1:4, :], cur_n[:, 1:4, cl : cl + BL]
                )
                nc.vector.tensor_mul(w[:, 0, :], nt[:, 0, :], omz[:, 0, :])
                if t + 1 < T:
                    nc.vector.tensor_add(hb[:, 0, :], w[:, 0, :], v[:, 0, :])
                nc.scalar.activation(
                    nt[:, 1:4, :],
                    pre_n[:, 1:4, :],
                    mybir.ActivationFunctionType.Tanh,
                )
                nc.vector.tensor_mul(w[:, 1:4, :], nt[:, 1:4, :], omz[:, 1:4, :])
                if t + 1 < T:
                    nc.vector.tensor_add(hb[:, 1:4, :], w[:, 1:4, :], v[:, 1:4, :])
                h_new = o_cur[:, :, ur, :]
                # fp32 h for output/next-step v, off the critical chain
                nc.gpsimd.tensor_add(h_new, w[:], v[:])
                h_prev = h_new

                if u == RT2 - 1:
                    cur = nxt
                    cur_n = nxt_n

                if ur == RT - 1:
                    nc.sync.dma_start(
                        ys_v[:, :, t - RT + 1 : t + 1, :], o_cur[:]
                    )

    nc.compile()
    return nc


def _get(T, use_bi, use_bhn):
    key = (T, use_bi, use_bhn)
    if key not in _cache:
        _cache[key] = _build(T, use_bi, use_bhn)
    return _cache[key]


def _pack_w(W, kc):
    # W [kc*128, M3*128] -> [128, M3*kc*128], col ((m*kc)+k)*128+c = W[k*128+p, m*128+c]
    return np.ascontiguousarray(
        W.astype(NPBF16).reshape(kc, 128, M3, 128).transpose(1, 2, 0, 3).reshape(128, -1)
    )


def kernel(x, h0, Wi, Wh, bi, bhn, _trace=False, _trace_kwargs=None):
    T = x.shape[0]
    use_bi = bool(np.any(bi))
    use_bhn = bool(np.any(bhn))
    nc = _get(T, use_bi, use_bhn)
    TB = T * BL

    wi_p = _pack_w(np.asarray(Wi), KD)
    wh_p = _pack_w(np.asarray(Wh), KH)
    x = np.asarray(x)
    h0 = np.asarray(h0)

    in_maps = []
    for c in range(NCORES):
        xc = x[:, c * BL : (c + 1) * BL, :]  # [T, BL, D]
        xt = np.ascontiguousarray(
            xc.astype(NPBF16).reshape(T, BL, KD, 128).transpose(3, 2, 0, 1).reshape(128, KD * TB)
        )
        h0c = np.ascontiguousarray(
            h0[c * BL : (c + 1) * BL, :].astype(np.float32).reshape(BL, KH, 128).transpose(2, 1, 0).reshape(128, KH * BL)
        )
        im = {"xt": xt, "wi": wi_p, "wh": wh_p, "h0t": h0c}
        if use_bi:
            im["bi_r"] = np.ascontiguousarray(bi.astype(NPBF16).reshape(1, M3 * 128))
        if use_bhn:
            im["bhn_t"] = np.ascontiguousarray(bhn.astype(np.float32).reshape(KH, 128).T)
        in_maps.append(im)

    kw = {}
    if _trace:
        kw = dict(trace=True, **(_trace_kwargs or {}))
    kernel._last_in_maps = in_maps
    res = bass_utils.run_bass_kernel_spmd(nc, in_maps, core_ids=list(range(NCORES)), **kw)

    ys = np.empty((T, B, H), dtype=np.float32)
    for c in range(NCORES):
        out = res.results[c]["yst"]  # [128, KH*TB]
        ys[:, c * BL : (c + 1) * BL, :] = (
            out.reshape(128, KH, T, BL).transpose(2, 3, 1, 0).reshape(T, BL, H)
        )
    kernel._last_result = res
    return ys
